# revision 7
# baseline (speedup 1.0000x reference)
"""Trainium2 Bass kernel for nn_LocalModel_Layer_35493609734520.

out[n] = sum_d x[n, d] * W[idx[n], d]   (gather row of W, dot with x row)

Strategy (data-parallel over N across 8 cores, 32768 rows/core):
  - Per 128-row tile, gather W rows via a one-hot matmul on TensorE:
      Wg = OH^T @ W  with OH^T[c, n] = (idx[n] == c).
    Everything is fp16 (x, W, one-hot): median rel err ~2.5e-4 vs the
    2e-2 gate, and fp16 halves both HBM traffic and matmul count
    (2 matmuls/tile instead of the 4 needed for bf16 hi/lo).
  - One-hot build: GpSimd only replicates the idx row across partitions
    (cheap); the is_equal runs on VectorE in 4x mode (16-bit SBUF).
  - ScalarE evicts each gather PSUM tile to SBUF fp16 so the VectorE
    x*Wg multiply-reduce runs in 2x fp16 mode with a fused accumulator.
  - Row layout n = p*256 + t keeps every DMA contiguous per partition
    (8KB x lines) and makes the out store a single 128x256 DMA.
"""

import numpy as np

N = 262144
D = 256
C = 256
NCORES = 8
NPC = N // NCORES  # 32768 rows per core
P = 128
TILES = NPC // P  # 256 tiles of 128 rows
GRP = 16  # tiles per block (idx broadcast / one-hot batch)
BLOCKS = TILES // GRP  # 16

_compiled = None


def _build(npc=NPC):
    import contextlib

    import concourse.bass as bass
    import concourse.mybir as mybir
    import concourse.tile as tile
    from concourse import bacc

    f16 = mybir.dt.float16
    f32 = mybir.dt.float32

    nc = bacc.Bacc("TRN2", target_bir_lowering=False, debug=False)

    x_d = nc.dram_tensor("x", [npc, D], f16, kind="ExternalInput").ap()
    # idx values as fp16 (0..255 exact), one row per block, j = g*128 + p
    idx_d = nc.dram_tensor("idx", [BLOCKS, GRP * P], f16, kind="ExternalInput").ap()
    w_d = nc.dram_tensor("W", [C, D], f16, kind="ExternalInput").ap()
    out_d = nc.dram_tensor("out", [npc, 1], f32, kind="ExternalOutput").ap()

    # row n = p*256 + t  (t = global tile id = b*GRP + g)
    x_view = x_d.rearrange("(p t) d -> p t d", p=P)  # [128, 256, 256]
    out_view = out_d.rearrange("(p t) one -> p (t one)", p=P)  # [128, 256]

    with tile.TileContext(nc) as tc:
        with contextlib.ExitStack() as ctx:
            const = ctx.enter_context(tc.tile_pool(name="const", bufs=1))
            xpool = ctx.enter_context(tc.tile_pool(name="xp", bufs=3))
            ipool = ctx.enter_context(tc.tile_pool(name="ip", bufs=2))
            rpool = ctx.enter_context(tc.tile_pool(name="rp", bufs=2))
            ohpool = ctx.enter_context(tc.tile_pool(name="oh", bufs=2))
            ppool = ctx.enter_context(tc.tile_pool(name="ps", bufs=8, space="PSUM"))
            wgpool = ctx.enter_context(tc.tile_pool(name="wg", bufs=4))
            spool = ctx.enter_context(tc.tile_pool(name="sp", bufs=4))
            opool = ctx.enter_context(tc.tile_pool(name="op", bufs=1))

            # ---- constants ----
            iota0 = const.tile([P, 1], f32, tag="iota0")
            nc.gpsimd.iota(
                iota0[:],
                pattern=[[0, 1]],
                base=0,
                channel_multiplier=1,
                allow_small_or_imprecise_dtypes=True,
            )
            iota1 = const.tile([P, 1], f32, tag="iota1")
            nc.gpsimd.iota(
                iota1[:],
                pattern=[[0, 1]],
                base=P,
                channel_multiplier=1,
                allow_small_or_imprecise_dtypes=True,
            )

            # W halves, fp16, loaded once
            w_sb = [
                const.tile([P, D], f16, tag=f"w{h}", name=f"w{h}") for h in range(2)
            ]
            for h in range(2):
                nc.sync.dma_start(w_sb[h][:], w_d[h * P : (h + 1) * P, :])

            out_sb = opool.tile([P, TILES], f32, tag="outsb")

            for b in range(BLOCKS):
                x_sb = xpool.tile([P, GRP, D], f16, tag="x")
                nc.sync.dma_start(x_sb[:], x_view[:, b * GRP : (b + 1) * GRP, :])

                idx_row = ipool.tile([1, GRP * P], f16, tag="irow")
                nc.sync.dma_start(idx_row[:], idx_d[b : b + 1, :])

                idx_rep = rpool.tile([P, GRP * P], f16, tag="irep")
                nc.gpsimd.partition_broadcast(idx_rep[:], idx_row[:])

                # one-hot^T halves: oh[c, j] = (idx[j] == c (+128)), fp16
                oh0 = ohpool.tile([P, GRP * P], f16, tag="oh0")
                oh1 = ohpool.tile([P, GRP * P], f16, tag="oh1")
                nc.vector.tensor_scalar(
                    oh0[:], idx_rep[:], iota0[:], None, op0=mybir.AluOpType.is_equal
                )
                nc.vector.tensor_scalar(
                    oh1[:], idx_rep[:], iota1[:], None, op0=mybir.AluOpType.is_equal
                )

                for g in range(GRP):
                    t_glob = b * GRP + g
                    ps = ppool.tile([P, D], f32, tag="psum")
                    nc.tensor.matmul(
                        ps[:],
                        oh0[:, g * P : (g + 1) * P],
                        w_sb[0][:],
                        start=True,
                        stop=False,
                    )
                    nc.tensor.matmul(
                        ps[:],
                        oh1[:, g * P : (g + 1) * P],
                        w_sb[1][:],
                        start=False,
                        stop=True,
                    )

                    # evict gather to SBUF fp16 on ScalarE (frees DVE 2x mode)
                    wg = wgpool.tile([P, D], f16, tag="wg")
                    nc.scalar.copy(wg[:], ps[:])

                    prod = spool.tile([P, D], f16, tag="prod")
                    nc.vector.scalar_tensor_tensor(
                        out=prod[:],
                        in0=x_sb[:, g, :],
                        scalar=1.0,
                        in1=wg[:],
                        op0=mybir.AluOpType.mult,
                        op1=mybir.AluOpType.mult,
                        accum_out=out_sb[:, t_glob : t_glob + 1],
                    )

            nc.sync.dma_start(out_view[:, :], out_sb[:])

    nc.compile()
    return nc


def _get_compiled():
    global _compiled
    if _compiled is None:
        _compiled = _build()
    return _compiled


def _make_in_maps(inputs):
    x16 = np.asarray(inputs["x"]).astype(np.float16)
    ids = np.asarray(inputs["idx"]).reshape(-1).astype(np.int64)
    w16 = np.ascontiguousarray(np.asarray(inputs["W"]).astype(np.float16))

    in_maps = []
    for c in range(NCORES):
        xs = np.ascontiguousarray(x16[c * NPC : (c + 1) * NPC])
        ids_core = ids[c * NPC : (c + 1) * NPC]
        # idx_staged[b, g*128 + p] = idx[p*256 + b*16 + g]
        ids2 = ids_core.reshape(P, BLOCKS, GRP)  # [p, b, g]
        staged = (
            ids2.transpose(1, 2, 0).reshape(BLOCKS, GRP * P).astype(np.float16)
        )
        in_maps.append({"x": xs, "idx": np.ascontiguousarray(staged), "W": w16})
    return in_maps


def kernel(x, idx, W):
    from concourse.bass_utils import run_bass_kernel_spmd

    nc = _get_compiled()
    in_maps = _make_in_maps({"x": x, "idx": idx, "W": W})
    res = run_bass_kernel_spmd(nc, in_maps, core_ids=list(range(NCORES)))
    out = np.concatenate([res.results[c]["out"] for c in range(NCORES)], axis=0)
    return out.reshape(N, 1).astype(np.float32)


# revision 9
# speedup vs baseline: 1.1582x; 1.1582x over previous
"""Trainium2 Bass kernel for nn_LocalModel_Layer_35493609734520.

out[n] = sum_d x[n, d] * W[idx[n], d]   (gather row of W, dot with x row)

Strategy (data-parallel over N across 8 cores, 32768 rows/core):
  - Per 128-row tile, gather W rows via a one-hot matmul on TensorE:
      Wg = OH^T @ W  with OH^T[c, n] = (idx[n] == c).
    Everything is fp16 (x, W, one-hot): median rel err ~2.5e-4 vs the
    2e-2 gate, and fp16 halves both HBM traffic and matmul count
    (2 matmuls/tile instead of the 4 needed for bf16 hi/lo).
  - One-hot build: GpSimd only replicates the idx row across partitions
    (cheap); the is_equal runs on VectorE in 4x mode (16-bit SBUF).
  - ScalarE evicts each gather PSUM tile to SBUF fp16 so the VectorE
    x*Wg multiply-reduce runs in 2x fp16 mode with a fused accumulator.
  - Row layout n = p*256 + t keeps every DMA contiguous per partition
    (8KB x lines) and makes the out store a single 128x256 DMA.
"""

import numpy as np

N = 262144
D = 256
C = 256
NCORES = 8
NPC = N // NCORES  # 32768 rows per core
P = 128
TILES = NPC // P  # 256 tiles of 128 rows
GRP = 16  # tiles per block (idx broadcast / one-hot batch)
BLOCKS = TILES // GRP  # 16

_compiled = None


def _build(npc=NPC):
    import contextlib

    import concourse.bass as bass
    import concourse.mybir as mybir
    import concourse.tile as tile
    from concourse import bacc

    f16 = mybir.dt.float16
    f32 = mybir.dt.float32

    nc = bacc.Bacc("TRN2", target_bir_lowering=False, debug=False)

    x_d = nc.dram_tensor("x", [npc, D], f16, kind="ExternalInput").ap()
    # idx values as fp16 (0..255 exact), one row per block, j = g*128 + p
    idx_d = nc.dram_tensor("idx", [BLOCKS, GRP * P], f16, kind="ExternalInput").ap()
    w_d = nc.dram_tensor("W", [C, D], f16, kind="ExternalInput").ap()
    out_d = nc.dram_tensor("out", [npc, 1], f32, kind="ExternalOutput").ap()

    # row n = p*256 + t  (t = global tile id = b*GRP + g)
    x_view = x_d.rearrange("(p t) d -> p t d", p=P)  # [128, 256, 256]
    out_view = out_d.rearrange("(p t) one -> p (t one)", p=P)  # [128, 256]

    with tile.TileContext(nc) as tc:
        with contextlib.ExitStack() as ctx:
            const = ctx.enter_context(tc.tile_pool(name="const", bufs=1))
            xpool = ctx.enter_context(tc.tile_pool(name="xp", bufs=3))
            ipool = ctx.enter_context(tc.tile_pool(name="ip", bufs=3))
            rpool = ctx.enter_context(tc.tile_pool(name="rp", bufs=3))
            ohpool = ctx.enter_context(tc.tile_pool(name="oh", bufs=3))
            ppool = ctx.enter_context(tc.tile_pool(name="ps", bufs=8, space="PSUM"))
            wgpool = ctx.enter_context(tc.tile_pool(name="wg", bufs=6))
            spool = ctx.enter_context(tc.tile_pool(name="sp", bufs=6))
            opool = ctx.enter_context(tc.tile_pool(name="op", bufs=1))

            # ---- constants ----
            iota0 = const.tile([P, 1], f32, tag="iota0")
            nc.gpsimd.iota(
                iota0[:],
                pattern=[[0, 1]],
                base=0,
                channel_multiplier=1,
                allow_small_or_imprecise_dtypes=True,
            )
            iota1 = const.tile([P, 1], f32, tag="iota1")
            nc.gpsimd.iota(
                iota1[:],
                pattern=[[0, 1]],
                base=P,
                channel_multiplier=1,
                allow_small_or_imprecise_dtypes=True,
            )

            # W halves, fp16, loaded once
            w_sb = [
                const.tile([P, D], f16, tag=f"w{h}", name=f"w{h}") for h in range(2)
            ]
            for h in range(2):
                nc.sync.dma_start(w_sb[h][:], w_d[h * P : (h + 1) * P, :])

            out_sb = opool.tile([P, TILES], f32, tag="outsb")

            for b in range(BLOCKS):
                x_sb = xpool.tile([P, GRP, D], f16, tag="x")
                nc.sync.dma_start(x_sb[:], x_view[:, b * GRP : (b + 1) * GRP, :])

                idx_row = ipool.tile([1, GRP * P], f16, tag="irow")
                nc.sync.dma_start(idx_row[:], idx_d[b : b + 1, :])

                idx_rep = rpool.tile([P, GRP * P], f16, tag="irep")
                nc.gpsimd.partition_broadcast(idx_rep[:], idx_row[:])

                # one-hot^T halves: oh[c, j] = (idx[j] == c (+128)), fp16
                oh0 = ohpool.tile([P, GRP * P], f16, tag="oh0")
                oh1 = ohpool.tile([P, GRP * P], f16, tag="oh1")
                nc.vector.tensor_scalar(
                    oh0[:], idx_rep[:], iota0[:], None, op0=mybir.AluOpType.is_equal
                )
                nc.vector.tensor_scalar(
                    oh1[:], idx_rep[:], iota1[:], None, op0=mybir.AluOpType.is_equal
                )

                # 2 tiles per PSUM bank: 16 gathers in flight, batched evict
                for g2 in range(GRP // 2):
                    ps = ppool.tile([P, 2 * D], f32, tag="psum")
                    for k in range(2):
                        g = g2 * 2 + k
                        sl = ps[:, k * D : (k + 1) * D]
                        nc.tensor.matmul(
                            sl,
                            oh0[:, g * P : (g + 1) * P],
                            w_sb[0][:],
                            start=True,
                            stop=False,
                        )
                        nc.tensor.matmul(
                            sl,
                            oh1[:, g * P : (g + 1) * P],
                            w_sb[1][:],
                            start=False,
                            stop=True,
                        )

                    # evict both gathers to SBUF fp16 on ScalarE in one op
                    wg = wgpool.tile([P, 2 * D], f16, tag="wg")
                    nc.scalar.copy(wg[:], ps[:])

                    for k in range(2):
                        g = g2 * 2 + k
                        t_glob = b * GRP + g
                        prod = spool.tile([P, D], f16, tag="prod")
                        nc.vector.scalar_tensor_tensor(
                            out=prod[:],
                            in0=x_sb[:, g, :],
                            scalar=1.0,
                            in1=wg[:, k * D : (k + 1) * D],
                            op0=mybir.AluOpType.mult,
                            op1=mybir.AluOpType.mult,
                            accum_out=out_sb[:, t_glob : t_glob + 1],
                        )

            nc.sync.dma_start(out_view[:, :], out_sb[:])

    nc.compile()
    return nc


def _get_compiled():
    global _compiled
    if _compiled is None:
        _compiled = _build()
    return _compiled


def _make_in_maps(inputs):
    x16 = np.asarray(inputs["x"]).astype(np.float16)
    ids = np.asarray(inputs["idx"]).reshape(-1).astype(np.int64)
    w16 = np.ascontiguousarray(np.asarray(inputs["W"]).astype(np.float16))

    in_maps = []
    for c in range(NCORES):
        xs = np.ascontiguousarray(x16[c * NPC : (c + 1) * NPC])
        ids_core = ids[c * NPC : (c + 1) * NPC]
        # idx_staged[b, g*128 + p] = idx[p*256 + b*16 + g]
        ids2 = ids_core.reshape(P, BLOCKS, GRP)  # [p, b, g]
        staged = (
            ids2.transpose(1, 2, 0).reshape(BLOCKS, GRP * P).astype(np.float16)
        )
        in_maps.append({"x": xs, "idx": np.ascontiguousarray(staged), "W": w16})
    return in_maps


def kernel(x, idx, W):
    from concourse.bass_utils import run_bass_kernel_spmd

    nc = _get_compiled()
    in_maps = _make_in_maps({"x": x, "idx": idx, "W": W})
    res = run_bass_kernel_spmd(nc, in_maps, core_ids=list(range(NCORES)))
    out = np.concatenate([res.results[c]["out"] for c in range(NCORES)], axis=0)
    return out.reshape(N, 1).astype(np.float32)
